# revision 47
# baseline (speedup 1.0000x reference)
"""Bass/Trainium2 kernel for nn_DiffAllocator (64x7 Sinkhorn, 200 iterations).

Raw-bass implementation (no Tile framework) with hand-rolled semaphores,
using the proven 4-op loop structure:

    r = A1 q   (PE matvec)   u = 1/r  (DVE reciprocal)
    c = A2 u   (PE matvec)   q = 1/c  (DVE reciprocal)

Iteration 1 (the only exact-LSE user) runs on host; iterations 2..200 run on
device in this multiplicative form anchored at stabilizers, re-anchored
("fold") every 16 iterations so no flushed-to-zero entry is ever remembered.
Every in-loop dependency is cross-engine (PE<->DVE semaphores) - the pattern
whose 536ns/iteration cost is hardware-validated. Same-engine DVE chaining is
NOT used (DVE SBUF writes drain ~60-125ns after the engine frees; a program-
order consumer races, as measured).

Why raw bass: the Tile framework routes fold work through the same per-engine
semaphore counters as the loop, so each fold's snapshot reads and PE
transposes stall the loop ~460ns (3 stalls/fold, ~5.6us total). With separate
semaphores per producer/consumer pair the folds run entirely in the loop's
shadow on Pool/ACT (+2 donated PE transpose slots), and the switch waits are
pre-satisfied. Intra-Pool dependencies rely on GPSIMD's in-order execution
(hardware-validated deterministic over repeated runs); every cross-engine
dependency carries a semaphore. The it=192 fold is dropped (the 24-iteration
tail drifts only ~e^30, far inside fp32 range) and the basis switch applies
DELAY=9 iterations after the snapshot (fp32-validated in emulation).

The semaphore region is cleared at program start behind an all-engine
barrier: the device may carry residual semaphore values from a previous NEFF
execution, which pre-satisfy waits and race the whole program (observed as
non-deterministic results; the Tile framework's preamble memsets exist for
the same reason).
"""

import numpy as np

L, B = 64, 7
EPS = 0.02
ITERS = 200
FOLD_EVERY = 16
LAST_FOLD = 176
DELAY = 9       # fold snapshot -> basis switch
PSIT_SLOT = 2   # fold + n: PE emits the psi transpose after this iteration
XT_SLOT = 6     # fold + n: PE emits the X transpose after this iteration

_CACHE = {}
DEBUG_DUMP = False


def _build_nc(reps=1):
    import contextlib
    import concourse.bacc as bacc
    import concourse.mybir as mybir

    f32 = mybir.dt.float32
    u32 = mybir.dt.uint32
    AF = mybir.ActivationFunctionType
    OP = mybir.AluOpType

    nc = bacc.Bacc("TRN2", target_bir_lowering=False, debug=False)

    # ---- DRAM I/O ----
    # WIN packs every input into one DMA:
    #   rows 0:7,  cols 0:68  = [A1_0 (64) | lb | 1/b | b | psi0]
    #   rows 0:64, cols 68:85 = [A2_0 (7) | K (7) | la | 1/a | pa0]
    d_WIN = nc.dram_tensor("WIN_in", [L, 85], f32, kind="ExternalInput").ap()
    d_id = nc.dram_tensor("ident_in", [L, L], f32, kind="ExternalInput").ap()
    d_P = nc.dram_tensor("P_out", [L, B], f32, kind="ExternalOutput").ap()
    dbg_specs = [("u0", [L, 1]), ("u1", [L, 1]), ("u2", [L, 1]), ("u3", [L, 1]),
                 ("q0", [B, 1]), ("q1", [B, 1]), ("q2", [B, 1]), ("q3", [B, 1]),
                 ("pa0", [L, 1]), ("pa1", [L, 1]), ("psi0", [B, 1]),
                 ("psi1", [B, 1]), ("A1_1", [B, L]), ("A2_1", [L, B]),
                 ("A1sw", [B, L]), ("cb", [B, 1]), ("T1", [L, B]),
                 ("X", [L, B]), ("Pu", [L, B]), ("bq", [B, 1]),
                 ("psi_r", [1, B]), ("psi_bc", [L, B]), ("lnu", [L, 1]),
                 ("lnq", [B, 1])]
    d_dbg = {}
    if DEBUG_DUMP:
        for nm, shp in dbg_specs:
            d_dbg[nm] = nc.dram_tensor(f"dbg_{nm}", shp, f32,
                                       kind="ExternalOutput").ap()

    n_end = 2 + (ITERS - 1) * reps
    iters = list(range(2, n_end))
    folds = [it for it in iters if it % FOLD_EVERY == 0 and it <= LAST_FOLD]
    fold_of = {it_f: f for f, it_f in enumerate(folds)}
    switch_of = {it_f + DELAY: f for f, it_f in enumerate(folds)}
    psiT_slot = {it_f + PSIT_SLOT: f for f, it_f in enumerate(folds)}
    xt_slot = {it_f + XT_SLOT: f for f, it_f in enumerate(folds)}
    war_slot = {it_f + 4: f for f, it_f in enumerate(folds)}
    n_folds = len(folds)
    final_epoch = n_folds % 2
    final_par = iters[-1] % 4
    k_last = len(iters)

    LN2 = float(np.log(2.0))
    C1, C2 = LN2 / (2.0 ** 23), -127.0 * LN2

    es = contextlib.ExitStack()
    with es:
        sb = lambda name, shape: es.enter_context(
            nc.sbuf_tensor(name, shape, f32))
        WIN = sb("WIN", [L, 85])
        ident = sb("ident", [L, L])
        A1_1 = sb("A1_1", [B, L])
        A2_1 = sb("A2_1", [L, B])
        A1sw = sb("A1sw", [B, L])
        qts = [sb(f"q{i}", [B, 1]) for i in range(4)]
        uts = [sb(f"u{i}", [L, 1]) for i in range(4)]
        pa_0 = sb("pa_0", [L, 1])
        psiC0 = sb("psiC0", [B, 1])
        psi_r = sb("psi_r", [1, B])
        psi_bc = sb("psi_bc", [L, B])
        T1 = sb("T1", [L, B])
        X = sb("X", [L, B])
        lnu = sb("lnu", [L, 1]); yfu = sb("yfu", [L, 1])
        lnq = sb("lnq", [B, 1]); yfq = sb("yfq", [B, 1])
        ncb = sb("ncb", [B, 1]); cb = sb("cb", [B, 1])
        Pu = sb("Pu", [L, B])
        bq = sb("bq", [B, 1])
        PT7 = sb("PT7", [B, L])
        scr7 = sb("scr7", [B, 1])

        psr = es.enter_context(nc.psum_tensor("psr", [L, 1], f32))
        psc = es.enter_context(nc.psum_tensor("psc", [B, 1], f32))
        psq = es.enter_context(nc.psum_tensor("psq", [1, B], f32))
        psa = es.enter_context(nc.psum_tensor("psa", [B, L], f32))

        sem = lambda name: es.enter_context(nc.semaphore(name))
        dsem = sem("dsem")          # W7 DMA
        dsemI = sem("dsemI")        # ident DMA
        pe_sem = sem("pe_sem")      # +2 per iteration (each matvec)
        dve_sem = sem("dve_sem")    # +2 per iteration (each reciprocal)
        poolA = sem("poolA")        # +1 per fold (stage A done)
        peT = sem("peT")            # +1 per fold (psi transpose done)
        actP = sem("actP")          # +1 per fold (psi row copy done)
        poolB = sem("poolB")        # +1 per fold (T1 ready)
        actA2 = sem("actA2")        # +1 per fold (A2 regen done)
        poolX = sem("poolX")        # +1 per fold (X ready)
        peXT = sem("peXT")          # +1 per fold (X transpose done)
        actA1 = sem("actA1")        # +1 per fold (A1 regen done)
        poolSW = sem("poolSW")      # +1 per fold (A1sw ready)
        poolF = sem("poolF")        # final: Pu ready
        dveB = sem("dveB")          # final: bq ready
        peF = sem("peF")            # final: P transpose ready
        dveF2 = sem("dveF2")        # final: PT7 ready

        lbc = WIN[0:B, L:L + 1]
        invb = WIN[0:B, L + 1:L + 2]
        bcol = WIN[0:B, L + 2:L + 3]
        psi1 = WIN[0:B, L + 3:L + 4]
        K = WIN[:, 75:82]
        la = WIN[:, 82:83]
        inva = WIN[:, 83:84]
        A1 = [WIN[0:B, 0:L], A1_1[:, :]]
        A2 = [WIN[:, 68:75], A2_1[:, :]]
        pa = [pa_0[:, :], WIN[:, 84:85]]
        psi_c = [psiC0[:, :], psi1]
        q = [t[:, :] for t in qts]
        up = [t[:, :] for t in uts]

        # The device may carry residual semaphore values from a previous NEFF
        # execution (the Tile framework clears the sem region in its preamble
        # for the same reason). Clear ours, then barrier before any waits.
        all_sems = [dsem, dsemI, pe_sem, dve_sem, poolA, peT, actP,
                    poolB, actA2, poolX, peXT, actA1, poolSW, poolF, dveB,
                    peF, dveF2]
        ids = [s.num for s in all_sems]
        nc.gpsimd.sem_clear(range(min(ids), max(ids) + 1))
        nc.all_engine_barrier()

        with nc.Block() as block:

            @block.sync
            def _(sync):
                nc.sync.dma_start(out=WIN[:, :], in_=d_WIN).then_inc(dsem, 16)
                nc.sync.dma_start(out=ident[:, :], in_=d_id).then_inc(dsemI, 16)
                with nc.allow_non_contiguous_dma(
                        reason="transposed 64x7 output, 1.8KB total"):
                    nc.sync.dma_start(
                        out=d_P.rearrange("a b -> b a"),
                        in_=PT7[:, :])._wait_ge(dveF2, 1).then_inc(dsem, 16)
                if DEBUG_DUMP:
                    srcs = {"u0": uts[0], "u1": uts[1], "u2": uts[2],
                            "u3": uts[3], "q0": qts[0], "q1": qts[1],
                            "q2": qts[2], "q3": qts[3], "pa0": pa_0,
                            "psi0": psiC0, "A1_1": A1_1, "A2_1": A2_1,
                            "A1sw": A1sw, "cb": cb, "T1": T1, "X": X,
                            "Pu": Pu, "bq": bq, "psi_r": psi_r,
                            "psi_bc": psi_bc, "lnu": lnu, "lnq": lnq}
                    for nm, srct in srcs.items():
                        nc.sync.dma_start(out=d_dbg[nm],
                                          in_=srct[:, :]).then_inc(dsem, 16)
                    nc.sync.dma_start(out=d_dbg["pa1"],
                                      in_=pa[1]).then_inc(dsem, 16)
                    nc.sync.dma_start(out=d_dbg["psi1"],
                                      in_=psi_c[1]).then_inc(dsem, 16)

            @block.tensor
            def _(te):
                epoch = 0
                for k, it in enumerate(iters, 1):
                    switching = it in switch_of
                    lhs1 = A1sw[:, :] if switching else A1[epoch]
                    lhs2 = A2[1 - epoch] if switching else A2[epoch]
                    q_in = invb if it == 2 else q[(it - 1) % 4]
                    if switching:
                        nc.tensor.wait_ge(poolSW, switch_of[it] + 1)
                    m1 = nc.tensor.matmul(psr[:, :], lhs1, q_in,
                                          start=True, stop=True)
                    if k > 1:
                        m1._wait_ge(dve_sem, 2 * (k - 1))
                    else:
                        m1._wait_ge(dsem, 16)
                    m1.then_inc(pe_sem)
                    m2 = nc.tensor.matmul(psc[:, :], lhs2, up[it % 4],
                                          start=True, stop=True)
                    m2._wait_ge(dve_sem, 2 * k - 1)
                    m2.then_inc(pe_sem)
                    if switching:
                        epoch = 1 - epoch
                    if it in psiT_slot:
                        f = psiT_slot[it]
                        fp = f % 2
                        if f == 0:
                            nc.tensor.wait_ge(dsemI, 16)   # ident DMA
                        else:
                            nc.tensor.wait_ge(actP, f)     # WAR: psq reuse
                        tp = nc.tensor.transpose(psq[:, :], psi_c[fp],
                                                 ident[0:B, 0:B])
                        tp._wait_ge(poolA, f + 1)
                        tp.then_inc(peT)
                    if it in xt_slot:
                        f = xt_slot[it]
                        if f > 0:
                            nc.tensor.wait_ge(actA1, f)    # WAR: psa reuse
                        t1 = nc.tensor.transpose(psa[:, :], X[:, :],
                                                 ident[:, :])
                        t1._wait_ge(poolX, f + 1)
                        t1.then_inc(peXT)
                # final: transpose Pu into psa
                nc.tensor.wait_ge(actA1, n_folds)   # WAR: psa vs last A1 copy
                tf = nc.tensor.transpose(psa[:, :], Pu[:, :], ident[:, :])
                tf._wait_ge(poolF, 1)
                tf.then_inc(peF)

            @block.vector
            def _(v):
                for k, it in enumerate(iters, 1):
                    par = it % 4
                    if it in war_slot:
                        nc.vector.wait_ge(poolA, war_slot[it] + 1)
                    r1 = nc.vector.reciprocal(up[par], psr[:, :])
                    r1._wait_ge(pe_sem, 2 * k - 1)
                    r1.then_inc(dve_sem)
                    r2 = nc.vector.reciprocal(q[par], psc[:, :])
                    r2._wait_ge(pe_sem, 2 * k)
                    r2.then_inc(dve_sem)
                # final: bq = q * b ; PT7 = psa * bq
                bqi = nc.vector.tensor_scalar(out=bq[:, :], in0=q[final_par],
                                              scalar1=bcol, scalar2=None,
                                              op0=OP.mult)
                bqi._wait_ge(dve_sem, 2 * k_last)
                bqi.then_inc(dveB)
                nc.vector.wait_ge(dveB, 1)
                pt = nc.vector.tensor_scalar(out=PT7[:, :], in0=psa[:, :],
                                             scalar1=bq[:, :], scalar2=None,
                                             op0=OP.mult)
                pt._wait_ge(peF, 1)
                pt.then_inc(dveF2)

            @block.scalar
            def _(s):
                nc.scalar.activation(scr7[:, :], lbc, AF.Exp)._wait_ge(dsem, 16)
                for f, it_f in enumerate(folds):
                    fp = f % 2
                    ne = 1 - (f % 2)
                    cbx = nc.scalar.activation(cb[:, :], ncb[:, :], AF.Exp)
                    cbx._wait_ge(poolA, f + 1)
                    pr = nc.scalar.copy(psi_r[:, :], psq[:, :])
                    pr._wait_ge(peT, f + 1)
                    pr.then_inc(actP)
                    a2x = nc.scalar.activation(A2[ne], T1[:, :], AF.Exp,
                                               bias=pa[fp])
                    a2x._wait_ge(poolB, f + 1)
                    a2x.then_inc(actA2)
                    a1c = nc.scalar.activation(A1[ne], psa[:, :], AF.Copy,
                                               scale=bcol)
                    a1c._wait_ge(peXT, f + 1)
                    a1c.then_inc(actA1)

            @block.gpsimd
            def _(g):
                for f, it_f in enumerate(folds):
                    par = it_f % 4
                    fp = f % 2
                    ne = 1 - (f % 2)
                    g1 = nc.gpsimd.tensor_copy(yfu[:, :], up[par].bitcast(u32))
                    g1._wait_ge(dve_sem, 2 * (it_f - 1))
                    nc.gpsimd.tensor_scalar(out=lnu[:, :], in0=yfu[:, :],
                                            scalar1=C1, scalar2=C2,
                                            op0=OP.mult, op1=OP.add)
                    nc.gpsimd.tensor_copy(yfq[:, :], q[par].bitcast(u32))
                    nc.gpsimd.tensor_scalar(out=lnq[:, :], in0=yfq[:, :],
                                            scalar1=C1, scalar2=C2,
                                            op0=OP.mult, op1=OP.add)
                    nc.gpsimd.tensor_scalar(out=pa[fp], in0=lnu[:, :],
                                            scalar1=pa[1 - fp], scalar2=la,
                                            op0=OP.add, op1=OP.add)
                    nc.gpsimd.tensor_scalar(out=psi_c[fp], in0=lnq[:, :],
                                            scalar1=psi_c[1 - fp], scalar2=lbc,
                                            op0=OP.add, op1=OP.add)
                    nc.gpsimd.tensor_scalar(out=ncb[:, :], in0=lnq[:, :],
                                            scalar1=lbc, scalar2=-1.0,
                                            op0=OP.add,
                                            op1=OP.mult).then_inc(poolA)
                    pb = nc.gpsimd.partition_broadcast(psi_bc[:, :],
                                                       psi_r[:, :])
                    pb._wait_ge(actP, f + 1)
                    nc.gpsimd.tensor_tensor(out=T1[:, :], in0=K,
                                            in1=psi_bc[:, :],
                                            op=OP.add).then_inc(poolB)
                    xx = nc.gpsimd.tensor_scalar(out=X[:, :], in0=A2[ne],
                                                 scalar1=inva, scalar2=None,
                                                 op0=OP.mult)
                    xx._wait_ge(actA2, f + 1)
                    xx.then_inc(poolX)
                    sw = nc.gpsimd.tensor_scalar(out=A1sw[:, :], in0=A1[ne],
                                                 scalar1=cb[:, :], scalar2=None,
                                                 op0=OP.mult)
                    sw._wait_ge(actA1, f + 1)
                    sw.then_inc(poolSW)
                # final: Pu = A2 * u
                pu = nc.gpsimd.tensor_scalar(out=Pu[:, :], in0=A2[final_epoch],
                                             scalar1=up[final_par],
                                             scalar2=None, op0=OP.mult)
                pu._wait_ge(dve_sem, 2 * k_last - 1)
                pu.then_inc(poolF)

        nc.compile()
    return nc


def _host_inputs(theta, phi, n, sens, err):
    f32 = np.float32
    theta = np.asarray(theta, f32); phi = np.asarray(phi, f32)
    n = np.asarray(n, f32); sens = np.asarray(sens, f32)
    err = np.asarray(err, f32)
    a = (n / n.sum()).astype(f32)
    e = np.exp((phi - phi.max()).astype(f32)); b = (e / e.sum()).astype(f32)
    C = ((n * sens)[:, None] * err[None, :]).astype(f32)
    K = ((theta - C) * f32(1.0 / EPS)).astype(f32)
    la = np.log(a).astype(f32)
    lb = np.log(b).astype(f32)

    # iteration 1 (log domain, max-stabilized LSE) + initial basis, on host
    def lse(x, axis):
        m = x.max(axis=axis, keepdims=True)
        return (m + np.log(np.exp(x - m).sum(axis=axis, keepdims=True))
                ).squeeze(axis).astype(f32)

    def ftz(x):
        x = np.asarray(x, f32).copy()
        x[np.abs(x) < 1.17549435e-38] = 0.0
        return x

    f1 = (la - lse(K, 1)).astype(f32)
    g1 = (lb - lse(K + f1[:, None], 0)).astype(f32)
    pa0 = (f1 + la).astype(f32)
    A2_0 = ftz(np.exp((K + pa0[:, None] + g1[None, :]).astype(f32)))
    A1_0 = ftz(ftz(A2_0 * (f32(1.0) / a)[:, None]).T * b[:, None])
    inva = (f32(1.0) / a).astype(f32)

    W7 = np.concatenate(
        [A1_0, np.stack([lb, f32(1.0) / b, b, g1], axis=1)], axis=1).astype(f32)
    WK = np.concatenate(
        [A2_0, K, np.stack([la, inva, pa0], axis=1)], axis=1).astype(f32)
    WIN = np.zeros((L, 85), f32)
    WIN[0:B, 0:L + 4] = W7
    WIN[:, 68:85] = WK
    return {
        "WIN_in": np.ascontiguousarray(WIN),
        "ident_in": np.eye(L, dtype=f32),
    }


def kernel(theta, phi, n, sens, err):
    if "nc" not in _CACHE:
        _CACHE["nc"] = _build_nc()
    nc = _CACHE["nc"]
    in_map = _host_inputs(theta, phi, n, sens, err)
    from concourse import bass_utils
    res = bass_utils.run_bass_kernel_spmd(nc, [in_map], [0])
    _CACHE["res"] = res
    return np.asarray(res.results[0]["P_out"], dtype=np.float32)


# revision 48
# speedup vs baseline: 1.0028x; 1.0028x over previous
"""Bass/Trainium2 kernel for nn_DiffAllocator (64x7 Sinkhorn, 200 iterations).

Raw-bass implementation (no Tile framework) with hand-rolled semaphores,
using the proven 4-op loop structure:

    r = A1 q   (PE matvec)   u = 1/r  (DVE reciprocal)
    c = A2 u   (PE matvec)   q = 1/c  (DVE reciprocal)

Iteration 1 (the only exact-LSE user) runs on host; iterations 2..200 run on
device in this multiplicative form anchored at stabilizers, re-anchored
("fold") every 16 iterations so no flushed-to-zero entry is ever remembered.
Every in-loop dependency is cross-engine (PE<->DVE semaphores) - the pattern
whose 536ns/iteration cost is hardware-validated. Same-engine DVE chaining is
NOT used (DVE SBUF writes drain ~60-125ns after the engine frees; a program-
order consumer races, as measured).

Why raw bass: the Tile framework routes fold work through the same per-engine
semaphore counters as the loop, so each fold's snapshot reads and PE
transposes stall the loop ~460ns (3 stalls/fold, ~5.6us total). With separate
semaphores per producer/consumer pair the folds run entirely in the loop's
shadow on Pool/ACT (+2 donated PE transpose slots), and the switch waits are
pre-satisfied. Intra-Pool dependencies rely on GPSIMD's in-order execution
(hardware-validated deterministic over repeated runs); every cross-engine
dependency carries a semaphore. The it=192 fold is dropped (the 24-iteration
tail drifts only ~e^30, far inside fp32 range) and the basis switch applies
DELAY=9 iterations after the snapshot (fp32-validated in emulation).

The semaphore region is cleared at program start behind an all-engine
barrier: the device may carry residual semaphore values from a previous NEFF
execution, which pre-satisfy waits and race the whole program (observed as
non-deterministic results; the Tile framework's preamble memsets exist for
the same reason).
"""

import numpy as np

L, B = 64, 7
EPS = 0.02
ITERS = 200
FOLD_EVERY = 16
LAST_FOLD = 176
DELAY = 9       # fold snapshot -> basis switch
PSIT_SLOT = 2   # fold + n: PE emits the psi transpose after this iteration
XT_SLOT = 6     # fold + n: PE emits the X transpose after this iteration

_CACHE = {}
DEBUG_DUMP = False


def _build_nc(reps=1):
    import contextlib
    import concourse.bacc as bacc
    import concourse.mybir as mybir

    f32 = mybir.dt.float32
    u32 = mybir.dt.uint32
    AF = mybir.ActivationFunctionType
    OP = mybir.AluOpType

    nc = bacc.Bacc("TRN2", target_bir_lowering=False, debug=False)

    # ---- DRAM I/O ----
    # WIN packs every input into one DMA:
    #   rows 0:7,  cols 0:68  = [A1_0 (64) | lb | 1/b | b | psi0]
    #   rows 0:64, cols 68:85 = [A2_0 (7) | K (7) | la | 1/a | pa0]
    d_WIN = nc.dram_tensor("WIN_in", [L, 85], f32, kind="ExternalInput").ap()
    d_id = nc.dram_tensor("ident_in", [L, L], f32, kind="ExternalInput").ap()
    d_P = nc.dram_tensor("P_out", [L, B], f32, kind="ExternalOutput").ap()
    dbg_specs = [("u0", [L, 1]), ("u1", [L, 1]), ("u2", [L, 1]), ("u3", [L, 1]),
                 ("q0", [B, 1]), ("q1", [B, 1]), ("q2", [B, 1]), ("q3", [B, 1]),
                 ("pa0", [L, 1]), ("pa1", [L, 1]), ("psi0", [B, 1]),
                 ("psi1", [B, 1]), ("A1_1", [B, L]), ("A2_1", [L, B]),
                 ("A1sw", [B, L]), ("cb", [B, 1]), ("T1", [L, B]),
                 ("X", [L, B]), ("Pu", [L, B]), ("bq", [B, 1]),
                 ("psi_r", [1, B]), ("psi_bc", [L, B]), ("lnu", [L, 1]),
                 ("lnq", [B, 1])]
    d_dbg = {}
    if DEBUG_DUMP:
        for nm, shp in dbg_specs:
            d_dbg[nm] = nc.dram_tensor(f"dbg_{nm}", shp, f32,
                                       kind="ExternalOutput").ap()

    n_end = 2 + (ITERS - 1) * reps
    iters = list(range(2, n_end))
    folds = [it for it in iters if it % FOLD_EVERY == 0 and it <= LAST_FOLD]
    fold_of = {it_f: f for f, it_f in enumerate(folds)}
    switch_of = {it_f + DELAY: f for f, it_f in enumerate(folds)}
    psiT_slot = {it_f + PSIT_SLOT: f for f, it_f in enumerate(folds)}
    xt_slot = {it_f + XT_SLOT: f for f, it_f in enumerate(folds)}
    war_slot = {it_f + 4: f for f, it_f in enumerate(folds)}
    n_folds = len(folds)
    final_epoch = n_folds % 2
    final_par = iters[-1] % 4
    k_last = len(iters)

    LN2 = float(np.log(2.0))
    C1, C2 = LN2 / (2.0 ** 23), -127.0 * LN2

    es = contextlib.ExitStack()
    with es:
        sb = lambda name, shape: es.enter_context(
            nc.sbuf_tensor(name, shape, f32))
        WIN = sb("WIN", [L, 85])
        ident = sb("ident", [L, L])
        A1_1 = sb("A1_1", [B, L])
        A2_1 = sb("A2_1", [L, B])
        A1sw = sb("A1sw", [B, L])
        qts = [sb(f"q{i}", [B, 1]) for i in range(4)]
        uts = [sb(f"u{i}", [L, 1]) for i in range(4)]
        pa_0 = sb("pa_0", [L, 1])
        psiC0 = sb("psiC0", [B, 1])
        psi_r = sb("psi_r", [1, B])
        psi_bc = sb("psi_bc", [L, B])
        T1 = sb("T1", [L, B])
        X = sb("X", [L, B])
        lnu = sb("lnu", [L, 1]); yfu = sb("yfu", [L, 1])
        lnq = sb("lnq", [B, 1]); yfq = sb("yfq", [B, 1])
        ncb = sb("ncb", [B, 1]); cb = sb("cb", [B, 1])
        Pu = sb("Pu", [L, B])
        bq = sb("bq", [B, 1])
        PT7 = sb("PT7", [B, L])
        scr7 = sb("scr7", [B, 1])

        psr = es.enter_context(nc.psum_tensor("psr", [L, 1], f32))
        psc = es.enter_context(nc.psum_tensor("psc", [B, 1], f32))
        psq = es.enter_context(nc.psum_tensor("psq", [1, B], f32))
        psa = es.enter_context(nc.psum_tensor("psa", [B, L], f32))

        sem = lambda name: es.enter_context(nc.semaphore(name))
        dsem = sem("dsem")          # W7 DMA
        dsemI = sem("dsemI")        # ident DMA
        pe_sem = sem("pe_sem")      # +2 per iteration (each matvec)
        dve_sem = sem("dve_sem")    # +2 per iteration (each reciprocal)
        poolA = sem("poolA")        # +1 per fold (stage A done)
        peT = sem("peT")            # +1 per fold (psi transpose done)
        actP = sem("actP")          # +1 per fold (psi row copy done)
        poolB = sem("poolB")        # +1 per fold (T1 ready)
        actA2 = sem("actA2")        # +1 per fold (A2 regen done)
        poolX = sem("poolX")        # +1 per fold (X ready)
        peXT = sem("peXT")          # +1 per fold (X transpose done)
        actA1 = sem("actA1")        # +1 per fold (A1 regen done)
        poolSW = sem("poolSW")      # +1 per fold (A1sw ready)
        poolF = sem("poolF")        # final: Pu ready
        dveB = sem("dveB")          # final: bq ready
        peF = sem("peF")            # final: P transpose ready
        dveF2 = sem("dveF2")        # final: PT7 ready

        lbc = WIN[0:B, L:L + 1]
        invb = WIN[0:B, L + 1:L + 2]
        bcol = WIN[0:B, L + 2:L + 3]
        psi1 = WIN[0:B, L + 3:L + 4]
        K = WIN[:, 75:82]
        la = WIN[:, 82:83]
        inva = WIN[:, 83:84]
        A1 = [WIN[0:B, 0:L], A1_1[:, :]]
        A2 = [WIN[:, 68:75], A2_1[:, :]]
        pa = [pa_0[:, :], WIN[:, 84:85]]
        psi_c = [psiC0[:, :], psi1]
        q = [t[:, :] for t in qts]
        up = [t[:, :] for t in uts]

        # The device may carry residual semaphore values from a previous NEFF
        # execution (the Tile framework clears the sem region in its preamble
        # for the same reason). Clear ours, then barrier before any waits.
        all_sems = [dsem, dsemI, pe_sem, dve_sem, poolA, peT, actP,
                    poolB, actA2, poolX, peXT, actA1, poolSW, poolF, dveB,
                    peF, dveF2]
        ids = [s.num for s in all_sems]
        nc.gpsimd.sem_clear(range(min(ids), max(ids) + 1))
        # Issue the input DMA before the boot barrier so its SEQ/HWDGE setup
        # overlaps the barrier. Safe vs the clear: the DMA's sem increment
        # lands ~3.1us into the run (fixed hardware path), the Pool clear by
        # ~0.7us; consumers are held at the barrier until the clear is done.
        nc.sync.dma_start(out=WIN[:, :], in_=d_WIN).then_inc(dsem, 16)
        nc.all_engine_barrier()

        with nc.Block() as block:

            @block.sync
            def _(sync):
                nc.sync.dma_start(out=ident[:, :], in_=d_id).then_inc(dsemI, 16)
                with nc.allow_non_contiguous_dma(
                        reason="transposed 64x7 output, 1.8KB total"):
                    nc.sync.dma_start(
                        out=d_P.rearrange("a b -> b a"),
                        in_=PT7[:, :])._wait_ge(dveF2, 1).then_inc(dsem, 16)
                if DEBUG_DUMP:
                    srcs = {"u0": uts[0], "u1": uts[1], "u2": uts[2],
                            "u3": uts[3], "q0": qts[0], "q1": qts[1],
                            "q2": qts[2], "q3": qts[3], "pa0": pa_0,
                            "psi0": psiC0, "A1_1": A1_1, "A2_1": A2_1,
                            "A1sw": A1sw, "cb": cb, "T1": T1, "X": X,
                            "Pu": Pu, "bq": bq, "psi_r": psi_r,
                            "psi_bc": psi_bc, "lnu": lnu, "lnq": lnq}
                    for nm, srct in srcs.items():
                        nc.sync.dma_start(out=d_dbg[nm],
                                          in_=srct[:, :]).then_inc(dsem, 16)
                    nc.sync.dma_start(out=d_dbg["pa1"],
                                      in_=pa[1]).then_inc(dsem, 16)
                    nc.sync.dma_start(out=d_dbg["psi1"],
                                      in_=psi_c[1]).then_inc(dsem, 16)

            @block.tensor
            def _(te):
                epoch = 0
                for k, it in enumerate(iters, 1):
                    switching = it in switch_of
                    lhs1 = A1sw[:, :] if switching else A1[epoch]
                    lhs2 = A2[1 - epoch] if switching else A2[epoch]
                    q_in = invb if it == 2 else q[(it - 1) % 4]
                    if switching:
                        nc.tensor.wait_ge(poolSW, switch_of[it] + 1)
                    m1 = nc.tensor.matmul(psr[:, :], lhs1, q_in,
                                          start=True, stop=True)
                    if k > 1:
                        m1._wait_ge(dve_sem, 2 * (k - 1))
                    else:
                        m1._wait_ge(dsem, 16)
                    m1.then_inc(pe_sem)
                    m2 = nc.tensor.matmul(psc[:, :], lhs2, up[it % 4],
                                          start=True, stop=True)
                    m2._wait_ge(dve_sem, 2 * k - 1)
                    m2.then_inc(pe_sem)
                    if switching:
                        epoch = 1 - epoch
                    if it in psiT_slot:
                        f = psiT_slot[it]
                        fp = f % 2
                        if f == 0:
                            nc.tensor.wait_ge(dsemI, 16)   # ident DMA
                        else:
                            nc.tensor.wait_ge(actP, f)     # WAR: psq reuse
                        tp = nc.tensor.transpose(psq[:, :], psi_c[fp],
                                                 ident[0:B, 0:B])
                        tp._wait_ge(poolA, f + 1)
                        tp.then_inc(peT)
                    if it in xt_slot:
                        f = xt_slot[it]
                        if f > 0:
                            nc.tensor.wait_ge(actA1, f)    # WAR: psa reuse
                        t1 = nc.tensor.transpose(psa[:, :], X[:, :],
                                                 ident[:, :])
                        t1._wait_ge(poolX, f + 1)
                        t1.then_inc(peXT)
                # final: transpose Pu into psa
                nc.tensor.wait_ge(actA1, n_folds)   # WAR: psa vs last A1 copy
                tf = nc.tensor.transpose(psa[:, :], Pu[:, :], ident[:, :])
                tf._wait_ge(poolF, 1)
                tf.then_inc(peF)

            @block.vector
            def _(v):
                for k, it in enumerate(iters, 1):
                    par = it % 4
                    if it in war_slot:
                        nc.vector.wait_ge(poolA, war_slot[it] + 1)
                    r1 = nc.vector.reciprocal(up[par], psr[:, :])
                    r1._wait_ge(pe_sem, 2 * k - 1)
                    r1.then_inc(dve_sem)
                    r2 = nc.vector.reciprocal(q[par], psc[:, :])
                    r2._wait_ge(pe_sem, 2 * k)
                    r2.then_inc(dve_sem)
                # final: bq = q * b ; PT7 = psa * bq
                bqi = nc.vector.tensor_scalar(out=bq[:, :], in0=q[final_par],
                                              scalar1=bcol, scalar2=None,
                                              op0=OP.mult)
                bqi._wait_ge(dve_sem, 2 * k_last)
                bqi.then_inc(dveB)
                nc.vector.wait_ge(dveB, 1)
                pt = nc.vector.tensor_scalar(out=PT7[:, :], in0=psa[:, :],
                                             scalar1=bq[:, :], scalar2=None,
                                             op0=OP.mult)
                pt._wait_ge(peF, 1)
                pt.then_inc(dveF2)

            @block.scalar
            def _(s):
                nc.scalar.activation(scr7[:, :], lbc, AF.Exp)._wait_ge(dsem, 16)
                for f, it_f in enumerate(folds):
                    fp = f % 2
                    ne = 1 - (f % 2)
                    cbx = nc.scalar.activation(cb[:, :], ncb[:, :], AF.Exp)
                    cbx._wait_ge(poolA, f + 1)
                    pr = nc.scalar.copy(psi_r[:, :], psq[:, :])
                    pr._wait_ge(peT, f + 1)
                    pr.then_inc(actP)
                    a2x = nc.scalar.activation(A2[ne], T1[:, :], AF.Exp,
                                               bias=pa[fp])
                    a2x._wait_ge(poolB, f + 1)
                    a2x.then_inc(actA2)
                    a1c = nc.scalar.activation(A1[ne], psa[:, :], AF.Copy,
                                               scale=bcol)
                    a1c._wait_ge(peXT, f + 1)
                    a1c.then_inc(actA1)

            @block.gpsimd
            def _(g):
                for f, it_f in enumerate(folds):
                    par = it_f % 4
                    fp = f % 2
                    ne = 1 - (f % 2)
                    g1 = nc.gpsimd.tensor_copy(yfu[:, :], up[par].bitcast(u32))
                    g1._wait_ge(dve_sem, 2 * (it_f - 1))
                    nc.gpsimd.tensor_scalar(out=lnu[:, :], in0=yfu[:, :],
                                            scalar1=C1, scalar2=C2,
                                            op0=OP.mult, op1=OP.add)
                    nc.gpsimd.tensor_copy(yfq[:, :], q[par].bitcast(u32))
                    nc.gpsimd.tensor_scalar(out=lnq[:, :], in0=yfq[:, :],
                                            scalar1=C1, scalar2=C2,
                                            op0=OP.mult, op1=OP.add)
                    nc.gpsimd.tensor_scalar(out=pa[fp], in0=lnu[:, :],
                                            scalar1=pa[1 - fp], scalar2=la,
                                            op0=OP.add, op1=OP.add)
                    nc.gpsimd.tensor_scalar(out=psi_c[fp], in0=lnq[:, :],
                                            scalar1=psi_c[1 - fp], scalar2=lbc,
                                            op0=OP.add, op1=OP.add)
                    nc.gpsimd.tensor_scalar(out=ncb[:, :], in0=lnq[:, :],
                                            scalar1=lbc, scalar2=-1.0,
                                            op0=OP.add,
                                            op1=OP.mult).then_inc(poolA)
                    pb = nc.gpsimd.partition_broadcast(psi_bc[:, :],
                                                       psi_r[:, :])
                    pb._wait_ge(actP, f + 1)
                    nc.gpsimd.tensor_tensor(out=T1[:, :], in0=K,
                                            in1=psi_bc[:, :],
                                            op=OP.add).then_inc(poolB)
                    xx = nc.gpsimd.tensor_scalar(out=X[:, :], in0=A2[ne],
                                                 scalar1=inva, scalar2=None,
                                                 op0=OP.mult)
                    xx._wait_ge(actA2, f + 1)
                    xx.then_inc(poolX)
                    sw = nc.gpsimd.tensor_scalar(out=A1sw[:, :], in0=A1[ne],
                                                 scalar1=cb[:, :], scalar2=None,
                                                 op0=OP.mult)
                    sw._wait_ge(actA1, f + 1)
                    sw.then_inc(poolSW)
                # final: Pu = A2 * u
                pu = nc.gpsimd.tensor_scalar(out=Pu[:, :], in0=A2[final_epoch],
                                             scalar1=up[final_par],
                                             scalar2=None, op0=OP.mult)
                pu._wait_ge(dve_sem, 2 * k_last - 1)
                pu.then_inc(poolF)

        nc.compile()
    return nc


def _host_inputs(theta, phi, n, sens, err):
    f32 = np.float32
    theta = np.asarray(theta, f32); phi = np.asarray(phi, f32)
    n = np.asarray(n, f32); sens = np.asarray(sens, f32)
    err = np.asarray(err, f32)
    a = (n / n.sum()).astype(f32)
    e = np.exp((phi - phi.max()).astype(f32)); b = (e / e.sum()).astype(f32)
    C = ((n * sens)[:, None] * err[None, :]).astype(f32)
    K = ((theta - C) * f32(1.0 / EPS)).astype(f32)
    la = np.log(a).astype(f32)
    lb = np.log(b).astype(f32)

    # iteration 1 (log domain, max-stabilized LSE) + initial basis, on host
    def lse(x, axis):
        m = x.max(axis=axis, keepdims=True)
        return (m + np.log(np.exp(x - m).sum(axis=axis, keepdims=True))
                ).squeeze(axis).astype(f32)

    def ftz(x):
        x = np.asarray(x, f32).copy()
        x[np.abs(x) < 1.17549435e-38] = 0.0
        return x

    f1 = (la - lse(K, 1)).astype(f32)
    g1 = (lb - lse(K + f1[:, None], 0)).astype(f32)
    pa0 = (f1 + la).astype(f32)
    A2_0 = ftz(np.exp((K + pa0[:, None] + g1[None, :]).astype(f32)))
    A1_0 = ftz(ftz(A2_0 * (f32(1.0) / a)[:, None]).T * b[:, None])
    inva = (f32(1.0) / a).astype(f32)

    W7 = np.concatenate(
        [A1_0, np.stack([lb, f32(1.0) / b, b, g1], axis=1)], axis=1).astype(f32)
    WK = np.concatenate(
        [A2_0, K, np.stack([la, inva, pa0], axis=1)], axis=1).astype(f32)
    WIN = np.zeros((L, 85), f32)
    WIN[0:B, 0:L + 4] = W7
    WIN[:, 68:85] = WK
    return {
        "WIN_in": np.ascontiguousarray(WIN),
        "ident_in": np.eye(L, dtype=f32),
    }


def kernel(theta, phi, n, sens, err):
    if "nc" not in _CACHE:
        _CACHE["nc"] = _build_nc()
    nc = _CACHE["nc"]
    in_map = _host_inputs(theta, phi, n, sens, err)
    from concourse import bass_utils
    res = bass_utils.run_bass_kernel_spmd(nc, [in_map], [0])
    _CACHE["res"] = res
    return np.asarray(res.results[0]["P_out"], dtype=np.float32)


# revision 49
# speedup vs baseline: 1.0030x; 1.0003x over previous
"""Bass/Trainium2 kernel for nn_DiffAllocator (64x7 Sinkhorn, 200 iterations).

Raw-bass implementation (no Tile framework) with hand-rolled semaphores,
using the proven 4-op loop structure:

    r = A1 q   (PE matvec)   u = 1/r  (DVE reciprocal)
    c = A2 u   (PE matvec)   q = 1/c  (DVE reciprocal)

Iteration 1 (the only exact-LSE user) runs on host; iterations 2..200 run on
device in this multiplicative form anchored at stabilizers, re-anchored
("fold") every 16 iterations so no flushed-to-zero entry is ever remembered.
Every in-loop dependency is cross-engine (PE<->DVE semaphores) - the pattern
whose 536ns/iteration cost is hardware-validated. Same-engine DVE chaining is
NOT used (DVE SBUF writes drain ~60-125ns after the engine frees; a program-
order consumer races, as measured).

Why raw bass: the Tile framework routes fold work through the same per-engine
semaphore counters as the loop, so each fold's snapshot reads and PE
transposes stall the loop ~460ns (3 stalls/fold, ~5.6us total). With separate
semaphores per producer/consumer pair the folds run entirely in the loop's
shadow on Pool/ACT (+2 donated PE transpose slots), and the switch waits are
pre-satisfied. Intra-Pool dependencies rely on GPSIMD's in-order execution
(hardware-validated deterministic over repeated runs); every cross-engine
dependency carries a semaphore. The it=192 fold is dropped (the 24-iteration
tail drifts only ~e^30, far inside fp32 range) and the basis switch applies
DELAY=9 iterations after the snapshot (fp32-validated in emulation).

The semaphore region is cleared at program start behind an all-engine
barrier: the device may carry residual semaphore values from a previous NEFF
execution, which pre-satisfy waits and race the whole program (observed as
non-deterministic results; the Tile framework's preamble memsets exist for
the same reason).
"""

import numpy as np

L, B = 64, 7
EPS = 0.02
ITERS = 200
FOLD_EVERY = 16
LAST_FOLD = 176
DELAY = 9       # fold snapshot -> basis switch
PSIT_SLOT = 2   # fold + n: PE emits the psi transpose after this iteration
XT_SLOT = 6     # fold + n: PE emits the X transpose after this iteration

_CACHE = {}
DEBUG_DUMP = False


def _build_nc(reps=1):
    import contextlib
    import concourse.bacc as bacc
    import concourse.mybir as mybir

    f32 = mybir.dt.float32
    u32 = mybir.dt.uint32
    AF = mybir.ActivationFunctionType
    OP = mybir.AluOpType

    nc = bacc.Bacc("TRN2", target_bir_lowering=False, debug=False)

    # ---- DRAM I/O ----
    # WIN packs every input into one DMA:
    #   rows 0:7,  cols 0:68  = [A1_0 (64) | lb | 1/b | b | psi0]
    #   rows 0:64, cols 68:85 = [A2_0 (7) | K (7) | la | 1/a | pa0]
    d_WIN = nc.dram_tensor("WIN_in", [L, 128], f32, kind="ExternalInput").ap()
    d_id = nc.dram_tensor("ident_in", [L, L], f32, kind="ExternalInput").ap()
    d_P = nc.dram_tensor("P_out", [L, B], f32, kind="ExternalOutput").ap()
    dbg_specs = [("u0", [L, 1]), ("u1", [L, 1]), ("u2", [L, 1]), ("u3", [L, 1]),
                 ("q0", [B, 1]), ("q1", [B, 1]), ("q2", [B, 1]), ("q3", [B, 1]),
                 ("pa0", [L, 1]), ("pa1", [L, 1]), ("psi0", [B, 1]),
                 ("psi1", [B, 1]), ("A1_1", [B, L]), ("A2_1", [L, B]),
                 ("A1sw", [B, L]), ("cb", [B, 1]), ("T1", [L, B]),
                 ("X", [L, B]), ("Pu", [L, B]), ("bq", [B, 1]),
                 ("psi_r", [1, B]), ("psi_bc", [L, B]), ("lnu", [L, 1]),
                 ("lnq", [B, 1])]
    d_dbg = {}
    if DEBUG_DUMP:
        for nm, shp in dbg_specs:
            d_dbg[nm] = nc.dram_tensor(f"dbg_{nm}", shp, f32,
                                       kind="ExternalOutput").ap()

    n_end = 2 + (ITERS - 1) * reps
    iters = list(range(2, n_end))
    folds = [it for it in iters if it % FOLD_EVERY == 0 and it <= LAST_FOLD]
    fold_of = {it_f: f for f, it_f in enumerate(folds)}
    switch_of = {it_f + DELAY: f for f, it_f in enumerate(folds)}
    psiT_slot = {it_f + PSIT_SLOT: f for f, it_f in enumerate(folds)}
    xt_slot = {it_f + XT_SLOT: f for f, it_f in enumerate(folds)}
    war_slot = {it_f + 4: f for f, it_f in enumerate(folds)}
    n_folds = len(folds)
    final_epoch = n_folds % 2
    final_par = iters[-1] % 4
    k_last = len(iters)

    LN2 = float(np.log(2.0))
    C1, C2 = LN2 / (2.0 ** 23), -127.0 * LN2

    es = contextlib.ExitStack()
    with es:
        sb = lambda name, shape: es.enter_context(
            nc.sbuf_tensor(name, shape, f32))
        WIN = sb("WIN", [L, 128])
        ident = sb("ident", [L, L])
        A1_1 = sb("A1_1", [B, L])
        A2_1 = sb("A2_1", [L, B])
        A1sw = sb("A1sw", [B, L])
        qts = [sb(f"q{i}", [B, 1]) for i in range(4)]
        uts = [sb(f"u{i}", [L, 1]) for i in range(4)]
        pa_0 = sb("pa_0", [L, 1])
        psiC0 = sb("psiC0", [B, 1])
        psi_r = sb("psi_r", [1, B])
        psi_bc = sb("psi_bc", [L, B])
        T1 = sb("T1", [L, B])
        X = sb("X", [L, B])
        lnu = sb("lnu", [L, 1]); yfu = sb("yfu", [L, 1])
        lnq = sb("lnq", [B, 1]); yfq = sb("yfq", [B, 1])
        ncb = sb("ncb", [B, 1]); cb = sb("cb", [B, 1])
        Pu = sb("Pu", [L, B])
        bq = sb("bq", [B, 1])
        PT7 = sb("PT7", [B, L])
        scr7 = sb("scr7", [B, 1])

        psr = es.enter_context(nc.psum_tensor("psr", [L, 1], f32))
        psc = es.enter_context(nc.psum_tensor("psc", [B, 1], f32))
        psq = es.enter_context(nc.psum_tensor("psq", [1, B], f32))
        psa = es.enter_context(nc.psum_tensor("psa", [B, L], f32))

        sem = lambda name: es.enter_context(nc.semaphore(name))
        dsem = sem("dsem")          # W7 DMA
        dsemI = sem("dsemI")        # ident DMA
        pe_sem = sem("pe_sem")      # +2 per iteration (each matvec)
        dve_sem = sem("dve_sem")    # +2 per iteration (each reciprocal)
        poolA = sem("poolA")        # +1 per fold (stage A done)
        peT = sem("peT")            # +1 per fold (psi transpose done)
        actP = sem("actP")          # +1 per fold (psi row copy done)
        poolB = sem("poolB")        # +1 per fold (T1 ready)
        actA2 = sem("actA2")        # +1 per fold (A2 regen done)
        poolX = sem("poolX")        # +1 per fold (X ready)
        peXT = sem("peXT")          # +1 per fold (X transpose done)
        actA1 = sem("actA1")        # +1 per fold (A1 regen done)
        poolSW = sem("poolSW")      # +1 per fold (A1sw ready)
        poolF = sem("poolF")        # final: Pu ready
        dveB = sem("dveB")          # final: bq ready
        peF = sem("peF")            # final: P transpose ready
        dveF2 = sem("dveF2")        # final: PT7 ready

        lbc = WIN[0:B, L:L + 1]
        invb = WIN[0:B, L + 1:L + 2]
        bcol = WIN[0:B, L + 2:L + 3]
        psi1 = WIN[0:B, L + 3:L + 4]
        K = WIN[:, 75:82]
        la = WIN[:, 82:83]
        inva = WIN[:, 83:84]
        A1 = [WIN[0:B, 0:L], A1_1[:, :]]
        A2 = [WIN[:, 68:75], A2_1[:, :]]
        pa = [pa_0[:, :], WIN[:, 84:85]]
        psi_c = [psiC0[:, :], psi1]
        q = [t[:, :] for t in qts]
        up = [t[:, :] for t in uts]

        # The device may carry residual semaphore values from a previous NEFF
        # execution (the Tile framework clears the sem region in its preamble
        # for the same reason). Clear ours, then barrier before any waits.
        all_sems = [dsem, dsemI, pe_sem, dve_sem, poolA, peT, actP,
                    poolB, actA2, poolX, peXT, actA1, poolSW, poolF, dveB,
                    peF, dveF2]
        ids = [s.num for s in all_sems]
        nc.gpsimd.sem_clear(range(min(ids), max(ids) + 1))
        # Issue the input DMA before the boot barrier so its SEQ/HWDGE setup
        # overlaps the barrier. Safe vs the clear: the DMA's sem increment
        # lands ~3.1us into the run (fixed hardware path), the Pool clear by
        # ~0.7us; consumers are held at the barrier until the clear is done.
        nc.sync.dma_start(out=WIN[:, :], in_=d_WIN).then_inc(dsem, 16)
        nc.all_engine_barrier()

        with nc.Block() as block:

            @block.sync
            def _(sync):
                nc.sync.dma_start(out=ident[:, :], in_=d_id).then_inc(dsemI, 16)
                with nc.allow_non_contiguous_dma(
                        reason="transposed 64x7 output, 1.8KB total"):
                    nc.sync.dma_start(
                        out=d_P.rearrange("a b -> b a"),
                        in_=PT7[:, :])._wait_ge(dveF2, 1).then_inc(dsem, 16)
                if DEBUG_DUMP:
                    srcs = {"u0": uts[0], "u1": uts[1], "u2": uts[2],
                            "u3": uts[3], "q0": qts[0], "q1": qts[1],
                            "q2": qts[2], "q3": qts[3], "pa0": pa_0,
                            "psi0": psiC0, "A1_1": A1_1, "A2_1": A2_1,
                            "A1sw": A1sw, "cb": cb, "T1": T1, "X": X,
                            "Pu": Pu, "bq": bq, "psi_r": psi_r,
                            "psi_bc": psi_bc, "lnu": lnu, "lnq": lnq}
                    for nm, srct in srcs.items():
                        nc.sync.dma_start(out=d_dbg[nm],
                                          in_=srct[:, :]).then_inc(dsem, 16)
                    nc.sync.dma_start(out=d_dbg["pa1"],
                                      in_=pa[1]).then_inc(dsem, 16)
                    nc.sync.dma_start(out=d_dbg["psi1"],
                                      in_=psi_c[1]).then_inc(dsem, 16)

            @block.tensor
            def _(te):
                epoch = 0
                for k, it in enumerate(iters, 1):
                    switching = it in switch_of
                    lhs1 = A1sw[:, :] if switching else A1[epoch]
                    lhs2 = A2[1 - epoch] if switching else A2[epoch]
                    q_in = invb if it == 2 else q[(it - 1) % 4]
                    if switching:
                        nc.tensor.wait_ge(poolSW, switch_of[it] + 1)
                    m1 = nc.tensor.matmul(psr[:, :], lhs1, q_in,
                                          start=True, stop=True)
                    if k > 1:
                        m1._wait_ge(dve_sem, 2 * (k - 1))
                    else:
                        m1._wait_ge(dsem, 16)
                    m1.then_inc(pe_sem)
                    m2 = nc.tensor.matmul(psc[:, :], lhs2, up[it % 4],
                                          start=True, stop=True)
                    m2._wait_ge(dve_sem, 2 * k - 1)
                    m2.then_inc(pe_sem)
                    if switching:
                        epoch = 1 - epoch
                    if it in psiT_slot:
                        f = psiT_slot[it]
                        fp = f % 2
                        if f == 0:
                            nc.tensor.wait_ge(dsemI, 16)   # ident DMA
                        else:
                            nc.tensor.wait_ge(actP, f)     # WAR: psq reuse
                        tp = nc.tensor.transpose(psq[:, :], psi_c[fp],
                                                 ident[0:B, 0:B])
                        tp._wait_ge(poolA, f + 1)
                        tp.then_inc(peT)
                    if it in xt_slot:
                        f = xt_slot[it]
                        if f > 0:
                            nc.tensor.wait_ge(actA1, f)    # WAR: psa reuse
                        t1 = nc.tensor.transpose(psa[:, :], X[:, :],
                                                 ident[:, :])
                        t1._wait_ge(poolX, f + 1)
                        t1.then_inc(peXT)
                # final: transpose Pu into psa
                nc.tensor.wait_ge(actA1, n_folds)   # WAR: psa vs last A1 copy
                tf = nc.tensor.transpose(psa[:, :], Pu[:, :], ident[:, :])
                tf._wait_ge(poolF, 1)
                tf.then_inc(peF)

            @block.vector
            def _(v):
                for k, it in enumerate(iters, 1):
                    par = it % 4
                    if it in war_slot:
                        nc.vector.wait_ge(poolA, war_slot[it] + 1)
                    r1 = nc.vector.reciprocal(up[par], psr[:, :])
                    r1._wait_ge(pe_sem, 2 * k - 1)
                    r1.then_inc(dve_sem)
                    r2 = nc.vector.reciprocal(q[par], psc[:, :])
                    r2._wait_ge(pe_sem, 2 * k)
                    r2.then_inc(dve_sem)
                # final: bq = q * b ; PT7 = psa * bq
                bqi = nc.vector.tensor_scalar(out=bq[:, :], in0=q[final_par],
                                              scalar1=bcol, scalar2=None,
                                              op0=OP.mult)
                bqi._wait_ge(dve_sem, 2 * k_last)
                bqi.then_inc(dveB)
                nc.vector.wait_ge(dveB, 1)
                pt = nc.vector.tensor_scalar(out=PT7[:, :], in0=psa[:, :],
                                             scalar1=bq[:, :], scalar2=None,
                                             op0=OP.mult)
                pt._wait_ge(peF, 1)
                pt.then_inc(dveF2)

            @block.scalar
            def _(s):
                nc.scalar.activation(scr7[:, :], lbc, AF.Exp)._wait_ge(dsem, 16)
                for f, it_f in enumerate(folds):
                    fp = f % 2
                    ne = 1 - (f % 2)
                    cbx = nc.scalar.activation(cb[:, :], ncb[:, :], AF.Exp)
                    cbx._wait_ge(poolA, f + 1)
                    pr = nc.scalar.copy(psi_r[:, :], psq[:, :])
                    pr._wait_ge(peT, f + 1)
                    pr.then_inc(actP)
                    a2x = nc.scalar.activation(A2[ne], T1[:, :], AF.Exp,
                                               bias=pa[fp])
                    a2x._wait_ge(poolB, f + 1)
                    a2x.then_inc(actA2)
                    a1c = nc.scalar.activation(A1[ne], psa[:, :], AF.Copy,
                                               scale=bcol)
                    a1c._wait_ge(peXT, f + 1)
                    a1c.then_inc(actA1)

            @block.gpsimd
            def _(g):
                for f, it_f in enumerate(folds):
                    par = it_f % 4
                    fp = f % 2
                    ne = 1 - (f % 2)
                    g1 = nc.gpsimd.tensor_copy(yfu[:, :], up[par].bitcast(u32))
                    g1._wait_ge(dve_sem, 2 * (it_f - 1))
                    nc.gpsimd.tensor_scalar(out=lnu[:, :], in0=yfu[:, :],
                                            scalar1=C1, scalar2=C2,
                                            op0=OP.mult, op1=OP.add)
                    nc.gpsimd.tensor_copy(yfq[:, :], q[par].bitcast(u32))
                    nc.gpsimd.tensor_scalar(out=lnq[:, :], in0=yfq[:, :],
                                            scalar1=C1, scalar2=C2,
                                            op0=OP.mult, op1=OP.add)
                    nc.gpsimd.tensor_scalar(out=pa[fp], in0=lnu[:, :],
                                            scalar1=pa[1 - fp], scalar2=la,
                                            op0=OP.add, op1=OP.add)
                    nc.gpsimd.tensor_scalar(out=psi_c[fp], in0=lnq[:, :],
                                            scalar1=psi_c[1 - fp], scalar2=lbc,
                                            op0=OP.add, op1=OP.add)
                    nc.gpsimd.tensor_scalar(out=ncb[:, :], in0=lnq[:, :],
                                            scalar1=lbc, scalar2=-1.0,
                                            op0=OP.add,
                                            op1=OP.mult).then_inc(poolA)
                    pb = nc.gpsimd.partition_broadcast(psi_bc[:, :],
                                                       psi_r[:, :])
                    pb._wait_ge(actP, f + 1)
                    nc.gpsimd.tensor_tensor(out=T1[:, :], in0=K,
                                            in1=psi_bc[:, :],
                                            op=OP.add).then_inc(poolB)
                    xx = nc.gpsimd.tensor_scalar(out=X[:, :], in0=A2[ne],
                                                 scalar1=inva, scalar2=None,
                                                 op0=OP.mult)
                    xx._wait_ge(actA2, f + 1)
                    xx.then_inc(poolX)
                    sw = nc.gpsimd.tensor_scalar(out=A1sw[:, :], in0=A1[ne],
                                                 scalar1=cb[:, :], scalar2=None,
                                                 op0=OP.mult)
                    sw._wait_ge(actA1, f + 1)
                    sw.then_inc(poolSW)
                # final: Pu = A2 * u
                pu = nc.gpsimd.tensor_scalar(out=Pu[:, :], in0=A2[final_epoch],
                                             scalar1=up[final_par],
                                             scalar2=None, op0=OP.mult)
                pu._wait_ge(dve_sem, 2 * k_last - 1)
                pu.then_inc(poolF)

        nc.compile()
    return nc


def _host_inputs(theta, phi, n, sens, err):
    f32 = np.float32
    theta = np.asarray(theta, f32); phi = np.asarray(phi, f32)
    n = np.asarray(n, f32); sens = np.asarray(sens, f32)
    err = np.asarray(err, f32)
    a = (n / n.sum()).astype(f32)
    e = np.exp((phi - phi.max()).astype(f32)); b = (e / e.sum()).astype(f32)
    C = ((n * sens)[:, None] * err[None, :]).astype(f32)
    K = ((theta - C) * f32(1.0 / EPS)).astype(f32)
    la = np.log(a).astype(f32)
    lb = np.log(b).astype(f32)

    # iteration 1 (log domain, max-stabilized LSE) + initial basis, on host
    def lse(x, axis):
        m = x.max(axis=axis, keepdims=True)
        return (m + np.log(np.exp(x - m).sum(axis=axis, keepdims=True))
                ).squeeze(axis).astype(f32)

    def ftz(x):
        x = np.asarray(x, f32).copy()
        x[np.abs(x) < 1.17549435e-38] = 0.0
        return x

    f1 = (la - lse(K, 1)).astype(f32)
    g1 = (lb - lse(K + f1[:, None], 0)).astype(f32)
    pa0 = (f1 + la).astype(f32)
    A2_0 = ftz(np.exp((K + pa0[:, None] + g1[None, :]).astype(f32)))
    A1_0 = ftz(ftz(A2_0 * (f32(1.0) / a)[:, None]).T * b[:, None])
    inva = (f32(1.0) / a).astype(f32)

    W7 = np.concatenate(
        [A1_0, np.stack([lb, f32(1.0) / b, b, g1], axis=1)], axis=1).astype(f32)
    WK = np.concatenate(
        [A2_0, K, np.stack([la, inva, pa0], axis=1)], axis=1).astype(f32)
    WIN = np.zeros((L, 128), f32)
    WIN[0:B, 0:L + 4] = W7
    WIN[:, 68:85] = WK
    return {
        "WIN_in": np.ascontiguousarray(WIN),
        "ident_in": np.eye(L, dtype=f32),
    }


def kernel(theta, phi, n, sens, err):
    if "nc" not in _CACHE:
        _CACHE["nc"] = _build_nc()
    nc = _CACHE["nc"]
    in_map = _host_inputs(theta, phi, n, sens, err)
    from concourse import bass_utils
    res = bass_utils.run_bass_kernel_spmd(nc, [in_map], [0])
    _CACHE["res"] = res
    return np.asarray(res.results[0]["P_out"], dtype=np.float32)
